# revision 22
# baseline (speedup 1.0000x reference)
"""Trainium2 Bass kernel for nn_CPSModel (SSGConv GNN + attention + ZINB decoder).

Strategy:
- Host: gcn_norm edge weights, edge sort/pad into per-core gather indices and
  selector matrices; project-then-propagate algebra (A^k(xW) = (A^k x)W) so
  propagation runs in 256 dims per scale instead of 2000.
- Device (8 cores, 1250 nodes/core padded to 1280): fused 3-scale conv
  projection, AllGather, 3 sparse propagation hops (indirect DMA gather +
  selector matmul segment-sum), BatchNorm via AllReduce, scale attention,
  ZINB decoder. All matmuls bf16 with f32 PSUM accumulation.
"""
import os
import sys
import types

sys.path.insert(0, '/opt/trn_rl_repo')

import numpy as np
import ml_dtypes

import concourse.bass as bass
import concourse.bacc as bacc
import concourse.tile as tile
from concourse import mybir
from concourse import bass_utils
from concourse.masks import make_identity

BF = mybir.dt.bfloat16
F32 = mybir.dt.float32
I32 = mybir.dt.int32
AL = mybir.AluOpType
ACT = mybir.ActivationFunctionType

NCORES = 8
N = 10000
SH = 1250          # nodes per core
SHP = 1280         # padded nodes per core
NT = SHP // 128    # node tiles per core
NP = SHP * NCORES  # padded global rows
HVGS = 2000
D = 256
D3 = 3 * D
HEADS = 8
HD = D // HEADS
ALPHA = 0.2
EPS = 1e-5
ENC = 128          # 2*FREQ fourier dims
HVGSP = 2048       # HVGS padded to a multiple of 128 for conv k-tiling

bf16 = ml_dtypes.bfloat16

_nc_cache = {}
last_exec_time_ns = None


# ---------------------------------------------------------------------------
# host preprocessing
# ---------------------------------------------------------------------------

def _pad_coord(n):
    """global node id -> padded row coordinate"""
    return (n // SH) * SHP + (n % SH)


def _edge_structures(edge_index):
    """Dense normalized adjacency, transposed & padded: AT[c, src_pad, dest_local]"""
    ei = np.asarray(edge_index).astype(np.int64)
    row = np.concatenate([ei[0], np.arange(N, dtype=np.int64)])
    col = np.concatenate([ei[1], np.arange(N, dtype=np.int64)])
    deg = np.bincount(col, minlength=N).astype(np.float32)
    dinv = np.where(deg > 0, 1.0 / np.sqrt(deg), 0.0).astype(np.float32)
    w = (dinv[row] * dinv[col]).astype(np.float32)

    core = col // SH
    dloc = col - core * SH
    src_pad = _pad_coord(row)
    at = np.zeros((NCORES, NP, SHP), np.float32)
    np.add.at(at, (core, src_pad, dloc), w)  # duplicate edges accumulate
    return at.astype(bf16)


def _wT(p, extra_bias=None):
    """[dout, din] linear params -> [din+1, dout] bf16 with bias as last row"""
    w = np.asarray(p["w"], np.float32)
    b = np.asarray(p["b"], np.float32) if extra_bias is None else extra_bias
    return np.concatenate([w.T, b[None, :]], 0).astype(bf16)


def _trivial(lnp):
    return (np.allclose(np.asarray(lnp["g"]), 1.0)
            and np.allclose(np.asarray(lnp["b"]), 0.0))


def _prep_inputs(x, edge_index, pos, params):
    x = np.asarray(x, np.float32)
    pos = np.asarray(pos, np.float32)
    tp, sp, dp = params["teacher"], params["student"], params["decoder"]

    at_all = _edge_structures(edge_index)

    # all layernorm/batchnorm affines in this model are identity (g=1, b=0)
    for lnp in [tp["ln"], sp["hid"][0]["ln"], sp["hid"][1]["ln"], dp["inp"]["ln"]] \
             + [tp["scales"][i]["bn"] for i in range(3)] \
             + [rb[k] for rb in dp["rb"] for k in ("ln1", "ln2")]:
        assert _trivial(lnp), "non-identity norm affine not supported"

    WcT = np.zeros((HVGSP, D3), bf16)
    WcT[:HVGS] = np.concatenate(
        [np.asarray(s["conv"]["w"], np.float32).T for s in tp["scales"]], 1
    ).astype(bf16)
    bconv = np.concatenate(
        [np.asarray(s["conv"]["b"], np.float32) for s in tp["scales"]]
    )[None, :].astype(np.float32)                                    # [1, 768]

    wq_mean = {
        "w": np.mean([np.asarray(s["q"]["w"], np.float32) for s in tp["scales"]], 0),
        "b": np.mean([np.asarray(s["q"]["b"], np.float32) for s in tp["scales"]], 0),
    }
    WqmT = _wT(wq_mean)                                              # [257, 256]
    WkvT = []
    for s in tp["scales"]:
        wk = np.asarray(s["k"]["w"], np.float32).T
        wv = np.asarray(s["v"]["w"], np.float32).T
        bk = np.asarray(s["k"]["b"], np.float32)
        bv = np.asarray(s["v"]["b"], np.float32)
        WkvT.append(np.concatenate(
            [np.concatenate([wk, wv], 1), np.concatenate([bk, bv])[None, :]], 0
        ).astype(bf16))                                              # [257, 512]
    WoutT = _wT(tp["out"])                                           # [257, 256]

    # student fourier features on host (trivial fraction of total FLOPs)
    sc = 2.0 * np.pi * (pos @ np.asarray(sp["B"], np.float32).T)
    enc = np.concatenate([np.cos(sc), np.sin(sc)], -1).astype(np.float32)  # [N, 128]
    Ws1T = _wT(sp["hid"][0]["lin"])                                  # [129, 256]
    Ws2T = _wT(sp["hid"][1]["lin"])                                  # [257, 256]
    Ws3T = _wT(sp["out"])                                            # [257, 256]

    WinpT = _wT(dp["inp"]["lin"])                                    # [257, 512]
    Wr = [[_wT(rb["lin1"]), _wT(rb["lin2"])] for rb in dp["rb"]]
    Wp0T = _wT(dp["proj"][0])                                        # [513, 1024]
    Wp1T = _wT(dp["proj"][1])                                        # [1025, 512]
    WmT = _wT(dp["mean"])                                            # [513, 2000]
    WdT = _wT(dp["disp"])
    WpT = _wT(dp["pi"])

    rowmask = np.zeros((128, NT), bf16)
    for t in range(NT):
        lo = t * 128
        valid = min(128, max(0, SH - lo))
        rowmask[:valid, t] = 1.0

    in_maps = []
    for c in range(NCORES):
        lo, hi = c * SH, (c + 1) * SH
        xT = np.zeros((HVGSP, SHP), bf16)
        xT[:HVGS, :SH] = x[lo:hi].T.astype(bf16)
        encT = np.zeros((ENC, SHP), bf16)
        encT[:, :SH] = enc[lo:hi].T.astype(bf16)
        m = {
            "xT": xT, "encT": encT,
            "AT": at_all[c],
            "rowmask": rowmask,
            "WcT": WcT, "bconv": bconv,
            "WqmT": WqmT, "Wkv0T": WkvT[0], "Wkv1T": WkvT[1], "Wkv2T": WkvT[2],
            "WoutT": WoutT,
            "Ws1T": Ws1T, "Ws2T": Ws2T, "Ws3T": Ws3T,
            "WinpT": WinpT,
            "Wr0aT": Wr[0][0], "Wr0bT": Wr[0][1],
            "Wr1aT": Wr[1][0], "Wr1bT": Wr[1][1],
            "Wr2aT": Wr[2][0], "Wr2bT": Wr[2][1],
            "Wp0T": Wp0T, "Wp1T": Wp1T,
            "WmT": WmT, "WdT": WdT, "WpT": WpT,
        }
        in_maps.append(m)
    return in_maps


# ---------------------------------------------------------------------------
# device graph
# ---------------------------------------------------------------------------

def _build():
    nc = bacc.Bacc("TRN2", target_bir_lowering=False, debug=False,
                   num_devices=NCORES)

    def din(name, shape, dt=BF):
        return nc.dram_tensor(name, shape, dt, kind="ExternalInput")

    xT = din("xT", [HVGSP, SHP])
    encT = din("encT", [ENC, SHP])
    AT = din("AT", [NP, SHP])
    rowmask = din("rowmask", [128, NT])
    WcT = din("WcT", [HVGSP, D3])
    bconv = din("bconv", [1, D3], F32)
    WqmT = din("WqmT", [D + 1, D])
    WkvT = [din(f"Wkv{i}T", [D + 1, 2 * D]) for i in range(3)]
    WoutT = din("WoutT", [D + 1, D])
    Ws1T = din("Ws1T", [ENC + 1, 256])
    Ws2T = din("Ws2T", [257, 256])
    Ws3T = din("Ws3T", [257, 256])
    WinpT = din("WinpT", [257, 512])
    WrT = [[din(f"Wr{i}aT", [dd + 1, dd]), din(f"Wr{i}bT", [dd + 1, dd])]
           for i, dd in enumerate([512, 1024, 512])]
    Wp0T = din("Wp0T", [513, 1024])
    Wp1T = din("Wp1T", [1025, 512])
    WmT = din("WmT", [513, 2000])
    WdT = din("WdT", [513, 2000])
    WpT = din("WpT", [513, 2000])

    out_zt = nc.dram_tensor("out_zt", [SHP, D], F32, kind="ExternalOutput")
    out_zs = nc.dram_tensor("out_zs", [SHP, D], F32, kind="ExternalOutput")
    out_mean = nc.dram_tensor("out_mean", [SHP, HVGS], F32, kind="ExternalOutput")
    out_disp = nc.dram_tensor("out_disp", [SHP, HVGS], F32, kind="ExternalOutput")
    out_pi = nc.dram_tensor("out_pi", [SHP, HVGS], F32, kind="ExternalOutput")
    out_attn = nc.dram_tensor("out_attn", [SHP, 24], F32, kind="ExternalOutput")

    dbg = bool(os.environ.get("KERNEL_DEBUG_TAPS"))
    if dbg:
        dbg_y = nc.dram_tensor("dbg_y", [SHP, D3], F32, kind="ExternalOutput")
        dbg_p1 = nc.dram_tensor("dbg_p1", [SHP, D], F32, kind="ExternalOutput")
        dbg_p1b = nc.dram_tensor("dbg_p1b", [SHP, 2 * D], F32, kind="ExternalOutput")
        dbg_p3 = nc.dram_tensor("dbg_p3", [SHP, D], F32, kind="ExternalOutput")
        dbg_hg = nc.dram_tensor("dbg_hg", [SHP, D3], F32, kind="ExternalOutput")
        dbg_bn = nc.dram_tensor("dbg_bn", [2, D3], F32, kind="ExternalOutput")

    # internal DRAM
    y_in = nc.dram_tensor("y_in", [SHP, D3], BF)
    y_full = nc.dram_tensor("y_full", [NP, D3], BF, addr_space="Shared")
    p1b_in = nc.dram_tensor("p1b_in", [SHP, 2 * D], BF)
    p1b_full = nc.dram_tensor("p1b_full", [NP, 2 * D], BF, addr_space="Shared")
    p2b_in = nc.dram_tensor("p2b_in", [SHP, D], BF)
    p2b_full = nc.dram_tensor("p2b_full", [NP, D], BF, addr_space="Shared")
    p1a = nc.dram_tensor("p1a", [SHP, D], BF)
    p2a = nc.dram_tensor("p2a", [SHP, D], BF)
    p3a = nc.dram_tensor("p3a", [SHP, D], BF)
    hg_d = nc.dram_tensor("hg_d", [SHP, D3], BF)
    bn_in = nc.dram_tensor("bn_in", [2, D3], F32)
    bn_out = nc.dram_tensor("bn_out", [2, D3], F32, addr_space="Shared")
    bn_apply = nc.dram_tensor("bn_apply", [2, D3], F32)

    RG = [list(range(NCORES))]

    with tile.TileContext(nc) as tc:
        with tc.tile_pool(name="const", bufs=1) as constp:
            ident = constp.tile([128, 128], BF, tag="ident")
            make_identity(nc, ident[:])
            ones1 = constp.tile([1, 128], BF, tag="ones1")
            nc.vector.memset(ones1[:], 1.0)
            mask_sb = constp.tile([128, NT], BF, tag="mask")
            nc.sync.dma_start(mask_sb[:], rowmask[:])
            eps_sb = constp.tile([128, 1], F32, tag="eps")
            nc.vector.memset(eps_sb[:], EPS)

            def rsqrt_dve(pool, pfx, var_ap, parts, free):
                """1/sqrt(var+eps) on DVE only: quake bit-trick + 2 Newton steps"""
                MAGIC1 = 0x5F3759DF + 1
                ve = pool.tile([parts, free], F32, tag=f"{pfx}ve", name=f"{pfx}ve")
                nc.vector.tensor_scalar(ve[:], var_ap, EPS, None, AL.add)
                t1 = pool.tile([parts, free], I32, tag=f"{pfx}t1", name=f"{pfx}t1")
                nc.vector.tensor_scalar(t1[:], ve[:].bitcast(I32), 1, None,
                                        AL.arith_shift_right)
                y0 = pool.tile([parts, free], F32, tag=f"{pfx}y0", name=f"{pfx}y0")
                nc.vector.tensor_scalar(t1[:], t1[:], -1, None, AL.bitwise_xor)
                nc.vector.tensor_scalar(y0[:].bitcast(I32), t1[:], MAGIC1, None,
                                        AL.add)
                sq = pool.tile([parts, free], F32, tag=f"{pfx}sq", name=f"{pfx}sq")
                for _ in range(2):
                    nc.vector.tensor_tensor(sq[:], y0[:], y0[:], op=AL.mult)
                    nc.vector.tensor_tensor(sq[:], sq[:], ve[:], op=AL.mult)
                    nc.vector.tensor_scalar(sq[:], sq[:], -0.5, 1.5, AL.mult, AL.add)
                    nc.vector.tensor_tensor(y0[:], y0[:], sq[:], op=AL.mult)
                return y0

            # ---------------- S0: conv projection y = x @ Wc^T ----------------
            with tc.tile_pool(name="convp", bufs=1) as convp, \
                 tc.tile_pool(name="convw", bufs=2) as convwk, \
                 tc.tile_pool(name="cpsum", bufs=2, space="PSUM") as cpsum:
                xt_sb = []
                for kc in range(HVGSP // 128):
                    t = convp.tile([128, SHP], BF, tag=f"xt{kc}")
                    nc.sync.dma_start(t[:], xT[kc * 128:(kc + 1) * 128, :])
                    xt_sb.append(t)
                wc_sb = []
                for kc in range(HVGSP // 128):
                    t = convp.tile([128, D3], BF, tag=f"wc{kc}")
                    nc.sync.dma_start(t[:], WcT[kc * 128:(kc + 1) * 128, :])
                    wc_sb.append(t)
                for nt in range(NT):
                    ps0 = cpsum.tile([128, 512], F32, tag="cps0", space="PSUM")
                    ps1 = cpsum.tile([128, 256], F32, tag="cps1", space="PSUM")
                    nk = HVGSP // 128
                    for kc in range(nk):
                        lhs = xt_sb[kc][:, nt * 128:(nt + 1) * 128]
                        nc.tensor.matmul(ps0[:], lhs, wc_sb[kc][:, 0:512],
                                         start=(kc == 0), stop=(kc == nk - 1))
                        nc.tensor.matmul(ps1[:], lhs, wc_sb[kc][:, 512:768],
                                         start=(kc == 0), stop=(kc == nk - 1))
                    ybf = convwk.tile([128, D3], BF, tag="ybf")
                    nc.scalar.copy(ybf[:, 0:512], ps0[:])
                    nc.scalar.copy(ybf[:, 512:768], ps1[:])
                    nc.sync.dma_start(y_in[nt * 128:(nt + 1) * 128, :], ybf[:])

            # ---------------- S1: AllGather y ----------------
            nc.gpsimd.collective_compute(
                "AllGather", AL.bypass, replica_groups=RG,
                ins=[y_in[:]], outs=[y_full[:]])

            # ---------------- student MLP (overlaps the hop phase) ----------------
            with tc.tile_pool(name="stw", bufs=1) as stw, \
                 tc.tile_pool(name="stp", bufs=2) as stp, \
                 tc.tile_pool(name="stmm", bufs=2, space="PSUM") as stmm, \
                 tc.tile_pool(name="sttr", bufs=2, space="PSUM") as sttr:

                def load_sw(handle, din_, dout, name):
                    tiles = []
                    for kc in range(din_ // 128):
                        t = stw.tile([128, dout], BF, tag=f"{name}k{kc}",
                                     name=f"{name}k{kc}")
                        nc.sync.dma_start(t[:], handle[kc * 128:(kc + 1) * 128, :])
                        tiles.append(t)
                    bias = stw.tile([1, dout], BF, tag=f"{name}b", name=f"{name}b")
                    nc.sync.dma_start(bias[:], handle[din_:din_ + 1, :])
                    return tiles, bias

                w_s1 = load_sw(Ws1T, ENC, 256, "s1")
                w_s2 = load_sw(Ws2T, 256, 256, "s2")
                w_s3 = load_sw(Ws3T, 256, 256, "s3")
                encs = stw.tile([128, SHP], BF, tag="encs")
                nc.sync.dma_start(encs[:], encT[:])

                def st_transpose(in_bf):
                    tr = stp.tile([128, 2, 128], BF, tag="sttr", name="sttr")
                    for kc in range(2):
                        pt = sttr.tile([128, 128], BF, tag="sttrp", name="sttrp",
                                       space="PSUM")
                        nc.tensor.transpose(
                            pt[:], in_bf[:, kc * 128:(kc + 1) * 128], ident[:])
                        nc.vector.tensor_copy(tr[:, kc, :], pt[:])
                    return tr

                def st_lnstats(cap):
                    st6 = stp.tile([128, 6], F32, tag="stst6", name="stst6")
                    nc.vector.bn_stats(st6[:], cap)
                    mv = stp.tile([128, 2], F32, tag="stmv", name="stmv")
                    nc.vector.bn_aggr(mv[:], st6[:])
                    invv = rsqrt_dve(stp, "stln", mv[:, 1:2], 128, 1)
                    return mv, invv

                for nt in range(NT):
                    sl = slice(nt * 128, (nt + 1) * 128)
                    sps = stmm.tile([128, 256], F32, tag="stmm", name="stmm",
                                    space="PSUM")
                    nc.tensor.matmul(sps[:], encs[:, sl], w_s1[0][0][:],
                                     start=True, stop=False)
                    nc.tensor.matmul(sps[:], ones1[:], w_s1[1][:], start=False,
                                     stop=True)
                    sst = stp.tile([128, 256], F32, tag="sst")
                    nc.scalar.activation(sst[:], sps[:], ACT.Silu)
                    mv, invv = st_lnstats(sst[:])
                    s1b = stp.tile([128, 256], BF, tag="s1b")
                    nc.vector.tensor_scalar(s1b[:], sst[:], mv[:, 0:1], invv[:, 0:1],
                                            AL.subtract, AL.mult)
                    s1T = st_transpose(s1b[:])
                    sps2 = stmm.tile([128, 256], F32, tag="stmm", name="stmm",
                                     space="PSUM")
                    for kc in range(2):
                        nc.tensor.matmul(sps2[:], s1T[:, kc, :], w_s2[0][kc][:],
                                         start=(kc == 0), stop=False)
                    nc.tensor.matmul(sps2[:], ones1[:], w_s2[1][:], start=False,
                                     stop=True)
                    sst2 = stp.tile([128, 256], F32, tag="sst2")
                    nc.scalar.activation(sst2[:], sps2[:], ACT.Silu)
                    mv, invv = st_lnstats(sst2[:])
                    s2b = stp.tile([128, 256], BF, tag="s2b")
                    nc.vector.tensor_scalar(s2b[:], sst2[:], mv[:, 0:1], invv[:, 0:1],
                                            AL.subtract, AL.mult)
                    s2T = st_transpose(s2b[:])
                    sps3 = stmm.tile([128, 256], F32, tag="stmm", name="stmm",
                                     space="PSUM")
                    for kc in range(2):
                        nc.tensor.matmul(sps3[:], s2T[:, kc, :], w_s3[0][kc][:],
                                         start=(kc == 0), stop=False)
                    nc.tensor.matmul(sps3[:], ones1[:], w_s3[1][:], start=False,
                                     stop=True)
                    zs_f = stp.tile([128, D], F32, tag="zs_f")
                    nc.scalar.copy(zs_f[:], sps3[:])
                    nc.sync.dma_start(out_zs[sl, :], zs_f[:])

            # ---------------- S2: propagation hops (dense A^T matmul) ----------------
            SLAB = 16
            NKT = NP // 128          # 80 source k-tiles
            NSLAB = NKT // SLAB

            def hop(cur_full, dims, evac):
                with tc.tile_pool(name="hopa", bufs=2) as ap_, \
                     tc.tile_pool(name="hopc", bufs=2) as cp_, \
                     tc.tile_pool(name="hopacc", bufs=1) as accp, \
                     tc.tile_pool(name="hopev", bufs=3) as evp, \
                     tc.tile_pool(name="hpsum", bufs=3, space="PSUM") as hp:
                    accs = []
                    for dt in range(NT):
                        a = accp.tile([128, dims], F32, tag=f"acc{dt}",
                                      name=f"acc{dt}")
                        accs.append(a)
                    for sb in range(NSLAB):
                        at_sb, cur_sb = [], []
                        for j in range(SLAB):
                            kt = sb * SLAB + j
                            t = ap_.tile([128, SHP], BF, tag=f"at{j}",
                                         name=f"at{j}")
                            (nc.sync, nc.scalar)[j % 2].dma_start(
                                t[:], AT[kt * 128:(kt + 1) * 128, :])
                            at_sb.append(t)
                            u = cp_.tile([128, dims], BF, tag=f"cur{j}",
                                         name=f"cur{j}")
                            (nc.scalar, nc.sync)[j % 2].dma_start(
                                u[:], cur_full[kt * 128:(kt + 1) * 128, :])
                            cur_sb.append(u)
                        for dt in range(NT):
                            ps0 = hp.tile([128, min(512, dims)], F32, tag="hA",
                                          name="hA", space="PSUM")
                            ps1 = None
                            if dims > 512:
                                ps1 = hp.tile([128, dims - 512], F32, tag="hB",
                                              name="hB", space="PSUM")
                            for j in range(SLAB):
                                lhs = at_sb[j][:, dt * 128:(dt + 1) * 128]
                                st, sp_ = (j == 0), (j == SLAB - 1)
                                nc.tensor.matmul(ps0[:], lhs,
                                                 cur_sb[j][:, 0:min(512, dims)],
                                                 start=st, stop=sp_)
                                if ps1 is not None:
                                    nc.tensor.matmul(ps1[:], lhs,
                                                     cur_sb[j][:, 512:dims],
                                                     start=st, stop=sp_)
                            if sb == 0:
                                nc.vector.tensor_copy(accs[dt][:, 0:min(512, dims)],
                                                      ps0[:])
                                if ps1 is not None:
                                    nc.vector.tensor_copy(accs[dt][:, 512:dims],
                                                          ps1[:])
                            else:
                                nc.vector.tensor_tensor(
                                    accs[dt][:, 0:min(512, dims)],
                                    accs[dt][:, 0:min(512, dims)], ps0[:], op=AL.add)
                                if ps1 is not None:
                                    nc.vector.tensor_tensor(
                                        accs[dt][:, 512:dims],
                                        accs[dt][:, 512:dims], ps1[:], op=AL.add)
                    for dt in range(NT):
                        evac(dt, accs[dt], evp)

            def evac2(dt, acc, pool):
                obf = pool.tile([128, 2 * D], BF, tag="ev2", name="ev2")
                nc.scalar.copy(obf[:], acc[:])
                sl = slice(dt * 128, (dt + 1) * 128)
                nc.sync.dma_start(p2a[sl, :], obf[:, 0:256])
                nc.sync.dma_start(p2b_in[sl, :], obf[:, 256:512])

            def evac3(dt, acc, pool):
                obf = pool.tile([128, D], BF, tag="ev3", name="ev3")
                nc.scalar.copy(obf[:], acc[:])
                nc.sync.dma_start(p3a[dt * 128:(dt + 1) * 128, :], obf[:])

            hop(y_full, D3, evac1)
            nc.gpsimd.collective_compute(
                "AllGather", AL.bypass, replica_groups=RG,
                ins=[p1b_in[:]], outs=[p1b_full[:]])
            hop(p1b_full, 2 * D, evac2)
            nc.gpsimd.collective_compute(
                "AllGather", AL.bypass, replica_groups=RG,
                ins=[p2b_in[:]], outs=[p2b_full[:]])
            hop(p2b_full, D, evac3)

            # ---------------- S3: combine + gelu + BN stats ----------------
            with tc.tile_pool(name="comb", bufs=3) as cb, \
                 tc.tile_pool(name="combc", bufs=1) as cbc, \
                 tc.tile_pool(name="spsum", bufs=1, space="PSUM") as spp:
                bias_bc = cbc.tile([128, D3], F32, tag="bias_bc")
                nc.sync.dma_start(bias_bc[:], bconv[:].to_broadcast([128, D3]))
                sA = spp.tile([1, 512], F32, tag="sA", space="PSUM")
                sB = spp.tile([1, 256], F32, tag="sB", space="PSUM")
                qA = spp.tile([1, 512], F32, tag="qA", space="PSUM")
                qB = spp.tile([1, 256], F32, tag="qB", space="PSUM")
                c1, c2, c3 = 0.8, 0.4, 0.8 / 3.0
                for nt in range(NT):
                    sl = slice(nt * 128, (nt + 1) * 128)
                    ym = cb.tile([128, D3], BF, tag="ym")
                    nc.sync.dma_start(ym[:], y_in[sl, :])
                    q1 = cb.tile([128, D], BF, tag="q1")
                    nc.sync.dma_start(q1[:], p1a[sl, :])
                    q1b = cb.tile([128, 2 * D], BF, tag="q1b")
                    nc.sync.dma_start(q1b[:], p1b_in[sl, :])
                    q2 = cb.tile([128, D], BF, tag="q2")
                    nc.sync.dma_start(q2[:], p2a[sl, :])
                    q2b = cb.tile([128, D], BF, tag="q2b")
                    nc.sync.dma_start(q2b[:], p2b_in[sl, :])
                    q3 = cb.tile([128, D], BF, tag="q3")
                    nc.sync.dma_start(q3[:], p3a[sl, :])

                    h = cb.tile([128, D3], F32, tag="h")
                    nc.vector.tensor_scalar(h[:], ym[:], ALPHA, None, AL.mult)
                    nc.vector.tensor_tensor(h[:], h[:], bias_bc[:], op=AL.add)
                    tmp = cb.tile([128, D], F32, tag="tmp")
                    # scale 1: + c1 * A y1
                    nc.vector.tensor_scalar(tmp[:], q1[:], c1, None, AL.mult)
                    nc.vector.tensor_tensor(h[:, 0:256], h[:, 0:256], tmp[:], op=AL.add)
                    # scale 2: + c2 * (A y2 + A^2 y2)
                    nc.vector.tensor_tensor(tmp[:], q1b[:, 0:256], q2[:], op=AL.add)
                    nc.vector.tensor_scalar(tmp[:], tmp[:], c2, None, AL.mult)
                    nc.vector.tensor_tensor(h[:, 256:512], h[:, 256:512], tmp[:], op=AL.add)
                    # scale 3: + c3 * (A y3 + A^2 y3 + A^3 y3)
                    nc.vector.tensor_tensor(tmp[:], q1b[:, 256:512], q2b[:], op=AL.add)
                    nc.vector.tensor_tensor(tmp[:], tmp[:], q3[:], op=AL.add)
                    nc.vector.tensor_scalar(tmp[:], tmp[:], c3, None, AL.mult)
                    nc.vector.tensor_tensor(h[:, 512:768], h[:, 512:768], tmp[:], op=AL.add)

                    hgb = cb.tile([128, D3], BF, tag="hgb")
                    nc.scalar.activation(hgb[:], h[:], ACT.Gelu)
                    nc.sync.dma_start(hg_d[sl, :], hgb[:])
                    hsq = cb.tile([128, D3], BF, tag="hsq")
                    nc.scalar.square(hsq[:], hgb[:])
                    st, sp_ = (nt == 0), (nt == NT - 1)
                    lhs = mask_sb[:, nt:nt + 1]
                    nc.tensor.matmul(sA[:], lhs, hgb[:, 0:512], start=st, stop=sp_)
                    nc.tensor.matmul(sB[:], lhs, hgb[:, 512:768], start=st, stop=sp_)
                    nc.tensor.matmul(qA[:], lhs, hsq[:, 0:512], start=st, stop=sp_)
                    nc.tensor.matmul(qB[:], lhs, hsq[:, 512:768], start=st, stop=sp_)
                stat_s = cbc.tile([1, D3], F32, tag="stat_s")
                nc.scalar.copy(stat_s[:, 0:512], sA[:])
                nc.scalar.copy(stat_s[:, 512:768], sB[:])
                stat_q = cbc.tile([1, D3], F32, tag="stat_q")
                nc.scalar.copy(stat_q[:, 0:512], qA[:])
                nc.scalar.copy(stat_q[:, 512:768], qB[:])
                nc.sync.dma_start(bn_in[0:1, :], stat_s[:])
                nc.sync.dma_start(bn_in[1:2, :], stat_q[:])

            nc.gpsimd.collective_compute(
                "AllReduce", AL.add, replica_groups=RG,
                ins=[bn_in[:]], outs=[bn_out[:]])

            # BN scale/shift from global stats
            with tc.tile_pool(name="bnp", bufs=1) as bnp:
                st_s = bnp.tile([1, D3], F32, tag="st_s")
                nc.sync.dma_start(st_s[:], bn_out[0:1, :])
                st_q = bnp.tile([1, D3], F32, tag="st_q")
                nc.sync.dma_start(st_q[:], bn_out[1:2, :])
                mean = bnp.tile([1, D3], F32, tag="mean")
                nc.scalar.mul(mean[:], st_s[:], 1.0 / N)
                msq = bnp.tile([1, D3], F32, tag="msq")
                nc.scalar.mul(msq[:], st_q[:], 1.0 / N)
                var = bnp.tile([1, D3], F32, tag="var")
                nc.vector.tensor_tensor(var[:], mean[:], mean[:], op=AL.mult)
                nc.vector.tensor_tensor(var[:], msq[:], var[:], op=AL.subtract)
                std = bnp.tile([1, D3], F32, tag="std")
                nc.scalar.activation(std[:], var[:], ACT.Sqrt, bias=eps_sb[0:1, 0:1])
                inv = bnp.tile([1, D3], F32, tag="inv")
                nc.vector.reciprocal(inv[:], std[:])
                shift = bnp.tile([1, D3], F32, tag="shift")
                nc.vector.tensor_tensor(shift[:], mean[:], inv[:], op=AL.mult)
                nc.scalar.mul(shift[:], shift[:], -1.0)
                nc.sync.dma_start(bn_apply[0:1, :], inv[:])
                nc.sync.dma_start(bn_apply[1:2, :], shift[:])

            # ---------------- S4: attention + decoder + student ----------------
            with tc.tile_pool(name="wp", bufs=1) as wp, \
                 tc.tile_pool(name="s4", bufs=3) as s4, \
                 tc.tile_pool(name="s4o", bufs=2) as s4o, \
                 tc.tile_pool(name="mmp", bufs=3, space="PSUM") as mmp, \
                 tc.tile_pool(name="trp", bufs=3, space="PSUM") as trp:

                inv_bc = wp.tile([128, D3], F32, tag="inv_bc")
                nc.gpsimd.dma_start(inv_bc[:], bn_apply[0:1, :].to_broadcast([128, D3]))
                shift_bc = wp.tile([128, D3], F32, tag="shift_bc")
                nc.gpsimd.dma_start(shift_bc[:], bn_apply[1:2, :].to_broadcast([128, D3]))

                def load_w(handle, din_, dout, name):
                    tiles = []
                    nk = din_ // 128
                    for kc in range(nk):
                        t = wp.tile([128, dout], BF, tag=f"{name}k{kc}")
                        nc.sync.dma_start(t[:], handle[kc * 128:(kc + 1) * 128, :])
                        tiles.append(t)
                    bias = wp.tile([1, dout], BF, tag=f"{name}b")
                    nc.sync.dma_start(bias[:], handle[din_:din_ + 1, :])
                    return tiles, bias

                w_qm = load_w(WqmT, D, D, "qm")
                w_kv = [load_w(WkvT[i], D, 2 * D, f"kv{i}") for i in range(3)]
                w_out = load_w(WoutT, D, D, "wout")
                w_s1 = load_w(Ws1T, ENC, 256, "s1")
                w_s2 = load_w(Ws2T, 256, 256, "s2")
                w_s3 = load_w(Ws3T, 256, 256, "s3")
                w_inp = load_w(WinpT, 256, 512, "inp")
                w_r = [[load_w(WrT[i][0], dd, dd, f"r{i}a"),
                        load_w(WrT[i][1], dd, dd, f"r{i}b")]
                       for i, dd in enumerate([512, 1024, 512])]
                w_p0 = load_w(Wp0T, 512, 1024, "p0")
                w_p1 = load_w(Wp1T, 1024, 512, "p1")
                w_m = load_w(WmT, 512, 2000, "wm")
                w_d = load_w(WdT, 512, 2000, "wd")
                w_pi = load_w(WpT, 512, 2000, "wpi")

                encs = wp.tile([128, SHP], BF, tag="encs")
                nc.sync.dma_start(encs[:], encT[:])

                def transpose_in(in_bf, din_, name):
                    """[128, din] bf16 -> [128, nk, 128] bf16 (transposed k-tiles)"""
                    nk = din_ // 128
                    tr = s4.tile([128, nk, 128], BF, tag=f"tr_{name}")
                    for kc in range(nk):
                        pt = trp.tile([128, 128], BF, tag="trp", space="PSUM")
                        nc.tensor.transpose(
                            pt[:], in_bf[:, kc * 128:(kc + 1) * 128], ident[:])
                        nc.scalar.copy(tr[:, kc, :], pt[:])
                    return tr

                def dense(in_bf, w, din_, dout, consumer, name):
                    tr = transpose_in(in_bf, din_, name)
                    tiles, bias = w
                    nk = din_ // 128
                    nchunk = (dout + 511) // 512
                    for oc in range(nchunk):
                        osz = min(512, dout - oc * 512)
                        osl = slice(oc * 512, oc * 512 + osz)
                        ps = mmp.tile([128, osz], F32, tag=f"mm{osz}", space="PSUM")
                        for kc in range(nk):
                            nc.tensor.matmul(ps[:], tr[:, kc, :], tiles[kc][:, osl],
                                             start=(kc == 0), stop=False)
                        nc.tensor.matmul(ps[:], ones1[:], bias[:, osl],
                                         start=False, stop=True)
                        consumer(ps, oc, osl, osz)

                def ln_stats(chunks, nchunk, name):
                    """chunks: list of [128, csz] aps; returns inv,[128,1] negmean"""
                    st6 = s4.tile([128, nchunk, 6], F32, tag=f"st6_{name}")
                    for i, cap in enumerate(chunks):
                        nc.vector.bn_stats(st6[:, i, :], cap)
                    mv = s4.tile([128, 2], F32, tag=f"mv_{name}")
                    nc.vector.bn_aggr(mv[:], st6[:])
                    stdv = s4.tile([128, 1], F32, tag=f"std_{name}")
                    nc.scalar.activation(stdv[:], mv[:, 1:2], ACT.Sqrt, bias=eps_sb[:, 0:1])
                    invv = s4.tile([128, 1], F32, tag=f"inv_{name}")
                    nc.vector.reciprocal(invv[:], stdv[:])
                    return mv, invv

                def lin_ln_silu(in_bf, w, din_, dout, out_tag, name):
                    """out = silu(LN(lin(in)))  -> [128, dout] bf16"""
                    stage = s4.tile([128, dout], F32, tag=f"stage{dout}")
                    def consume(ps, oc, osl, osz):
                        nc.scalar.copy(stage[:, osl], ps[:])
                    dense(in_bf, w, din_, dout, consume, name)
                    nchunk = (dout + 511) // 512
                    chunks = [stage[:, i * 512:min(dout, (i + 1) * 512)]
                              for i in range(nchunk)]
                    mv, invv = ln_stats(chunks, nchunk, name)
                    out_bf = s4o.tile([128, dout], BF, tag=out_tag)
                    nc.vector.tensor_scalar(stage[:], stage[:], mv[:, 0:1],
                                            invv[:, 0:1], AL.subtract, AL.mult)
                    nc.scalar.activation(out_bf[:], stage[:], ACT.Silu)
                    return out_bf

                for nt in range(NT):
                    sl = slice(nt * 128, (nt + 1) * 128)
                    # ---- batchnorm apply ----
                    hgm = s4.tile([128, D3], BF, tag="hgm")
                    nc.sync.dma_start(hgm[:], hg_d[sl, :])
                    hbn = s4.tile([128, D3], F32, tag="hbn")
                    nc.vector.tensor_tensor(hbn[:], hgm[:], inv_bc[:], op=AL.mult)
                    nc.vector.tensor_tensor(hbn[:], hbn[:], shift_bc[:], op=AL.add)
                    # ---- layernorm over each scale's 256 dims ----
                    nf = s4.tile([128, D3], BF, tag="nf")
                    for s in range(3):
                        ssl = slice(s * 256, (s + 1) * 256)
                        mv, invv = ln_stats([hbn[:, ssl]], 1, f"nf{s}")
                        nc.vector.tensor_scalar(nf[:, ssl], hbn[:, ssl], mv[:, 0:1],
                                                invv[:, 0:1], AL.subtract, AL.mult)
                    # ---- attention ----
                    nfT = transpose_in(nf[:], D3, "nf")
                    qps = mmp.tile([128, 256], F32, tag="mm256q", space="PSUM")
                    for kc in range(2):
                        nc.tensor.matmul(qps[:], nfT[:, kc, :], w_qm[0][kc][:],
                                         start=(kc == 0), stop=False)
                    nc.tensor.matmul(qps[:], ones1[:], w_qm[1][:], start=False, stop=True)
                    q_sb = s4.tile([128, D], F32, tag="q_sb")
                    nc.scalar.copy(q_sb[:], qps[:])
                    kv_sb = s4.tile([128, 3, 2 * D], BF, tag="kv_sb", bufs=2)
                    for s in range(3):
                        kvps = mmp.tile([128, 512], F32, tag="mm512kv", space="PSUM")
                        for kc in range(2):
                            nc.tensor.matmul(kvps[:], nfT[:, 2 * s + kc, :],
                                             w_kv[s][0][kc][:],
                                             start=(kc == 0), stop=False)
                        nc.tensor.matmul(kvps[:], ones1[:], w_kv[s][1][:],
                                         start=False, stop=True)
                        nc.scalar.copy(kv_sb[:, s, :], kvps[:])
                    scores = s4.tile([128, 3, 8], F32, tag="scores")
                    prod = s4.tile([128, D], F32, tag="prod")
                    for s in range(3):
                        nc.vector.tensor_tensor(prod[:], q_sb[:],
                                                kv_sb[:, s, 0:256], op=AL.mult)
                        nc.vector.tensor_reduce(
                            scores[:, s, :], prod[:].rearrange("p (h d) -> p h d", h=8),
                            axis=mybir.AxisListType.X, op=AL.add)
                    smax = s4.tile([128, 8], F32, tag="smax")
                    nc.vector.tensor_tensor(smax[:], scores[:, 0, :], scores[:, 1, :],
                                            op=AL.max)
                    nc.vector.tensor_tensor(smax[:], smax[:], scores[:, 2, :], op=AL.max)
                    ee = s4.tile([128, 3, 8], F32, tag="ee")
                    for s in range(3):
                        nc.vector.tensor_tensor(ee[:, s, :], scores[:, s, :], smax[:],
                                                op=AL.subtract)
                    nc.scalar.activation(ee[:], ee[:], ACT.Exp,
                                         scale=1.0 / np.sqrt(HD))
                    den = s4.tile([128, 8], F32, tag="den")
                    nc.vector.tensor_tensor(den[:], ee[:, 0, :], ee[:, 1, :], op=AL.add)
                    nc.vector.tensor_tensor(den[:], den[:], ee[:, 2, :], op=AL.add)
                    rden = s4.tile([128, 8], F32, tag="rden")
                    nc.vector.reciprocal(rden[:], den[:])
                    attnw = s4.tile([128, 3, 8], F32, tag="attnw")
                    for s in range(3):
                        nc.vector.tensor_tensor(attnw[:, s, :], ee[:, s, :], rden[:],
                                                op=AL.mult)
                    nc.sync.dma_start(out_attn[sl, :],
                                      attnw[:].rearrange("p a b -> p (a b)"))
                    att = s4.tile([128, D], F32, tag="att")
                    attv = att[:].rearrange("p (h d) -> p h d", h=8)
                    tmp2 = s4.tile([128, D], F32, tag="tmp2")
                    tmp2v = tmp2[:].rearrange("p (h d) -> p h d", h=8)
                    for s in range(3):
                        vv = kv_sb[:, s, 256:512].rearrange("p (h d) -> p h d", h=8)
                        bw = attnw[:, s, :].unsqueeze(-1).to_broadcast([128, 8, 32])
                        if s == 0:
                            nc.vector.tensor_tensor(attv, vv, bw, op=AL.mult)
                        else:
                            nc.vector.tensor_tensor(tmp2v, vv, bw, op=AL.mult)
                            nc.vector.tensor_tensor(att[:], att[:], tmp2[:], op=AL.add)
                    nc.vector.tensor_tensor(att[:], att[:], hbn[:, 0:256], op=AL.add)
                    att_bf = s4.tile([128, D], BF, tag="att_bf")
                    nc.vector.tensor_copy(att_bf[:], att[:])
                    # z_t = lin(out, att)
                    atT = transpose_in(att_bf[:], D, "att")
                    zps = mmp.tile([128, 256], F32, tag="mm256z", space="PSUM")
                    for kc in range(2):
                        nc.tensor.matmul(zps[:], atT[:, kc, :], w_out[0][kc][:],
                                         start=(kc == 0), stop=False)
                    nc.tensor.matmul(zps[:], ones1[:], w_out[1][:], start=False, stop=True)
                    zt_f = s4.tile([128, D], F32, tag="zt_f")
                    nc.scalar.copy(zt_f[:], zps[:])
                    nc.sync.dma_start(out_zt[sl, :], zt_f[:])
                    zt_bf = s4.tile([128, D], BF, tag="zt_bf")
                    nc.vector.tensor_copy(zt_bf[:], zps[:])

                    # ---- decoder ----
                    d0 = lin_ln_silu(zt_bf[:], w_inp, 256, 512, "d0", f"inp{nt}")
                    t1 = lin_ln_silu(d0[:], w_r[0][0], 512, 512, "t1", f"r0a{nt}")
                    t2 = lin_ln_silu(t1[:], w_r[0][1], 512, 512, "t2", f"r0b{nt}")
                    h1 = s4o.tile([128, 512], BF, tag="h1")
                    nc.vector.tensor_tensor(h1[:], t2[:], d0[:], op=AL.add)
                    h2 = s4o.tile([128, 1024], BF, tag="h2")
                    def cons_h2(ps, oc, osl, osz):
                        nc.scalar.copy(h2[:, osl], ps[:])
                    dense(h1[:], w_p0, 512, 1024, cons_h2, f"p0{nt}")
                    t3 = lin_ln_silu(h2[:], w_r[1][0], 1024, 1024, "t3", f"r1a{nt}")
                    t4 = lin_ln_silu(t3[:], w_r[1][1], 1024, 1024, "t4", f"r1b{nt}")
                    h3 = s4o.tile([128, 1024], BF, tag="h3")
                    nc.vector.tensor_tensor(h3[:], t4[:], h2[:], op=AL.add)
                    h4 = s4o.tile([128, 512], BF, tag="h4")
                    def cons_h4(ps, oc, osl, osz):
                        nc.scalar.copy(h4[:, osl], ps[:])
                    dense(h3[:], w_p1, 1024, 512, cons_h4, f"p1{nt}")
                    t5 = lin_ln_silu(h4[:], w_r[2][0], 512, 512, "t5", f"r2a{nt}")
                    t6 = lin_ln_silu(t5[:], w_r[2][1], 512, 512, "t6", f"r2b{nt}")
                    h5 = s4o.tile([128, 512], BF, tag="h5")
                    nc.vector.tensor_tensor(h5[:], t6[:], h4[:], op=AL.add)

                    h5T = transpose_in(h5[:], 512, "h5")
                    for w_h, act, dest in ((w_m, ACT.Softplus, out_mean),
                                           (w_d, ACT.Softplus, out_disp),
                                           (w_pi, ACT.Sigmoid, out_pi)):
                        for oc in range(4):
                            osl = slice(oc * 500, (oc + 1) * 500)
                            ps = mmp.tile([128, 500], F32, tag="mm500", space="PSUM")
                            for kc in range(4):
                                nc.tensor.matmul(ps[:], h5T[:, kc, :],
                                                 w_h[0][kc][:, osl],
                                                 start=(kc == 0), stop=False)
                            nc.tensor.matmul(ps[:], ones1[:], w_h[1][:, osl],
                                             start=False, stop=True)
                            ho = s4o.tile([128, 500], F32, tag="ho")
                            nc.scalar.activation(ho[:], ps[:], act)
                            nc.sync.dma_start(dest[sl, osl], ho[:])

                    # ---- student ----
                    sps = mmp.tile([128, 256], F32, tag="mm256s", space="PSUM")
                    nc.tensor.matmul(sps[:], encs[:, sl], w_s1[0][0][:],
                                     start=True, stop=False)
                    nc.tensor.matmul(sps[:], ones1[:], w_s1[1][:], start=False, stop=True)
                    sst = s4.tile([128, 256], F32, tag="sst")
                    nc.scalar.activation(sst[:], sps[:], ACT.Silu)
                    mv, invv = ln_stats([sst[:]], 1, "sln1")
                    s1b = s4.tile([128, 256], BF, tag="s1b")
                    nc.vector.tensor_scalar(s1b[:], sst[:], mv[:, 0:1], invv[:, 0:1],
                                            AL.subtract, AL.mult)
                    s1T = transpose_in(s1b[:], 256, "s1")
                    sps2 = mmp.tile([128, 256], F32, tag="mm256s2", space="PSUM")
                    for kc in range(2):
                        nc.tensor.matmul(sps2[:], s1T[:, kc, :], w_s2[0][kc][:],
                                         start=(kc == 0), stop=False)
                    nc.tensor.matmul(sps2[:], ones1[:], w_s2[1][:], start=False, stop=True)
                    nc.scalar.activation(sst[:], sps2[:], ACT.Silu)
                    mv, invv = ln_stats([sst[:]], 1, "sln2")
                    s2b = s4.tile([128, 256], BF, tag="s2b")
                    nc.vector.tensor_scalar(s2b[:], sst[:], mv[:, 0:1], invv[:, 0:1],
                                            AL.subtract, AL.mult)
                    s2T = transpose_in(s2b[:], 256, "s2")
                    sps3 = mmp.tile([128, 256], F32, tag="mm256s3", space="PSUM")
                    for kc in range(2):
                        nc.tensor.matmul(sps3[:], s2T[:, kc, :], w_s3[0][kc][:],
                                         start=(kc == 0), stop=False)
                    nc.tensor.matmul(sps3[:], ones1[:], w_s3[1][:], start=False, stop=True)
                    zs_f = s4.tile([128, D], F32, tag="zs_f")
                    nc.scalar.copy(zs_f[:], sps3[:])
                    nc.sync.dma_start(out_zs[sl, :], zs_f[:])

    nc.compile()
    return nc


# ---------------------------------------------------------------------------
# entry point
# ---------------------------------------------------------------------------

def _install_profile_hook():
    try:
        from trn_agent_boot.trn_boot import _ntff_profile_via_ctypes
        hook = _ntff_profile_via_ctypes('/opt/axon/libaxon_pjrt.so')
        if hook is None:
            return False
        mod = types.ModuleType('antenv.axon_hooks')
        mod.get_axon_ntff_profile_hook = lambda: hook
        sys.modules['antenv.axon_hooks'] = mod
        return True
    except Exception:
        return False


def kernel(x, edge_index, pos, params):
    global last_exec_time_ns
    in_maps = _prep_inputs(x, edge_index, pos, params)
    if "nc" not in _nc_cache:
        _nc_cache["nc"] = _build()
    nc = _nc_cache["nc"]

    trace = bool(os.environ.get("KERNEL_TRACE"))
    if trace:
        trace = _install_profile_hook()
    res = bass_utils.run_bass_kernel_spmd(
        nc, in_maps, core_ids=list(range(NCORES)), trace=trace)
    last_exec_time_ns = res.exec_time_ns

    def gather(name):
        return np.concatenate([res.results[c][name][:SH] for c in range(NCORES)], 0)

    z_t = gather("out_zt")
    z_s = gather("out_zs")
    mean = gather("out_mean")
    disp = gather("out_disp")
    pi = gather("out_pi")
    attn = gather("out_attn").reshape(N, 3, 8)
    return z_t, z_s, mean, disp, pi, attn


if __name__ == "__main__":
    import pickle
    with open('/root/problem/cache/all.pkl', 'rb') as fh:
        dd = pickle.load(fh)
    inputs, expected = dd['inputs'], dd['expected']
    os.environ.setdefault("KERNEL_TRACE", "1")
    out = kernel(**inputs)
    names = ["z_t", "z_s", "mean", "disp", "pi", "attn_w"]
    for nm, a, b in zip(names, out, expected):
        a = np.asarray(a, np.float32); b = np.asarray(b, np.float32)
        e = np.linalg.norm(a - b) / (np.linalg.norm(b) + 1e-12)
        print(f"{nm:8s} rel_err = {e:.3e}")
    print("HW exec time:", last_exec_time_ns, "ns")
